# revision 43
# baseline (speedup 1.0000x reference)
"""Multi-head self-attention (B=1, S=4096, D=2048, H=16, Dh=128) on 8 TRN2
NeuronCores. Head-sharded tensor parallelism: each core computes 2 heads end to
end in transposed layout, writes its partial out-projection [D, S] as fp16; the
host sums the 8 partials and transposes back to [S, D].

Dtype strategy: activations/weights stream as bf16 (matmul inputs), all matmul
accumulation is fp32 in PSUM.  q/k after rmsnorm are fp16 (same matmul speed,
more mantissa).  Attention scores are computed in S^T layout [k, q] so the
softmax reduction is a ones-column matmul (partition sum) and no transposes are
needed anywhere; the inter-tile sum-exp accumulator is bf16 so every vector add
runs in the DVE's packed 2x mode.

Schedule: phase 1 runs one output group (q0/k0/q1/k1/v) at a time over all 16
k-tiles of a resident x chunk, so each group's PSUM eviction + rmsnorm chain
hides under the next group's matmuls.  Phase 2 interleaves the previous q-block
out-projection into the attention loop; all PSUM evictions in the attention
loop go through the vector engine so the scalar engine does nothing but exp.
"""
import sys
import numpy as np

for _p in ("/opt/trn_rl_repo",):
    if _p not in sys.path:
        sys.path.append(_p)

import concourse.bacc as bacc
import concourse.mybir as mybir
import concourse.tile as tile

F32 = mybir.dt.float32
F32R = mybir.dt.float32r
F16 = mybir.dt.float16
BF16 = mybir.dt.bfloat16
AF = mybir.ActivationFunctionType
MUL = mybir.AluOpType.mult

D = 2048            # d_model
S = 4096            # sequence length
DH = 128            # head dim
HPC = 2             # heads per core
DHC = HPC * DH      # 256 head-dims per core
NC = 8              # cores
EPS = 1e-6
SCALE = 1.0 / np.sqrt(DH)

NCH = S // 512      # 8 seq chunks of 512
KT_D = D // 128     # 16 k-tiles over d_model
KT_S = S // 128     # 32 k-tiles over sequence

TRACE = False       # set by test harness for profiling runs


def build():
    nc = bacc.Bacc("TRN2", target_bir_lowering=False, debug=False)

    # x packed on host as [(n p), (kt c)] so one chunk is a single
    # contiguous 8KB/partition DMA
    xb = nc.dram_tensor("xb", [NCH * 128, KT_D * 512], BF16,
                        kind="ExternalInput")
    wqb = nc.dram_tensor("wqb", [128, KT_D * DHC], BF16, kind="ExternalInput")
    wkb = nc.dram_tensor("wkb", [128, KT_D * DHC], BF16, kind="ExternalInput")
    wvb = nc.dram_tensor("wvb", [128, KT_D * DHC], BF16, kind="ExternalInput")
    wob = nc.dram_tensor("wob", [128, HPC * D], BF16, kind="ExternalInput")
    qw = nc.dram_tensor("qw", [DH, 1], F32, kind="ExternalInput")
    kw = nc.dram_tensor("kw", [DH, 1], F32, kind="ExternalInput")
    ones_c_d = nc.dram_tensor("ones_c", [128, 1], F32, kind="ExternalInput")
    outT = nc.dram_tensor("outT", [D, S], F16, kind="ExternalOutput")

    xb_t = xb.rearrange("(n p) r -> n p r", p=128)          # [8,128,8192]
    outT_t = outT.rearrange("(mo p) s -> mo p s", p=128)    # [16,128,4096]

    with tile.TileContext(nc) as tc, \
         nc.allow_low_precision(reason="bf16/fp16/f32r compute is intentional"):
        with (
            tc.tile_pool(name="consts", bufs=1) as consts,
            tc.tile_pool(name="big", bufs=1) as big,
            tc.tile_pool(name="stream", bufs=6) as stream,
            tc.tile_pool(name="ev", bufs=1) as ev,
        ):
            # ---- residents ----
            ones_col = consts.tile([128, 1], F32R)         # lhsT for partition-sum
            ones_col_b = consts.tile([128, 1], BF16, tag="onesb")
            nc.sync.dma_start(out=ones_col[:], in_=ones_c_d[:].bitcast(F32R))
            nc.vector.memset(ones_col_b[:], 1.0)
            eps_sb = consts.tile([1, 1], F32, tag="eps")
            nc.vector.memset(eps_sb[:], EPS)
            qw_sb = consts.tile([DH, 1], F32, tag="qw")    # per-partition norm w
            kw_sb = consts.tile([DH, 1], F32, tag="kw")
            nc.sync.dma_start(out=qw_sb[:], in_=qw[:])
            nc.sync.dma_start(out=kw_sb[:], in_=kw[:])

            qT = [big.tile([128, S], F16, tag=f"q{h}", name=f"qT{h}")
                  for h in range(HPC)]
            kT = [big.tile([128, S], F16, tag=f"k{h}", name=f"kT{h}")
                  for h in range(HPC)]
            v_sb = big.tile([128, KT_S, DHC], BF16, tag="v")
            o_sb = [big.tile([128, S], BF16, tag=f"o{h}", name=f"o{h}")
                    for h in range(HPC)]
            wo_sb = big.tile([128, HPC, D], BF16, tag="wo")

            # ========== Phase 1: q/k/v projections + q/k rmsnorm ==========
            # x chunk resident in SBUF; one output group at a time over all
            # 16 k-tiles so each group's eviction chain overlaps the next
            # group's matmuls.
            with (
                tc.tile_pool(name="wqk", bufs=1) as wqk,
                tc.tile_pool(name="ps1", bufs=1, space="PSUM") as ps1,
            ):
                # first x chunk before the weights so the first matmul can
                # start as soon as x(0)+wq land; quarter-granular pieces so
                # the kt=0 matmuls only wait on the first piece
                def dma_x(x_tile, n, pieces=range(4)):
                    xf = x_tile[:].rearrange("p a b -> p (a b)")
                    for i in pieces:
                        qs = slice(i * 2048, (i + 1) * 2048)
                        nc.sync.dma_start(out=xf[:, qs], in_=xb_t[n][:, qs])

                x_cur = stream.tile([128, KT_D, 512], BF16, tag="x", bufs=2)
                wq_sb = wqk.tile([128, KT_D, DHC], BF16, tag="wq")
                wk_sb = wqk.tile([128, KT_D, DHC], BF16, tag="wk")
                wv_sb = wqk.tile([128, KT_D, DHC], BF16, tag="wv")
                dma_x(x_cur, 0, pieces=(0,))
                wqf = wq_sb[:].rearrange("p a b -> p (a b)")
                nc.sync.dma_start(out=wqf[:, 0:1024], in_=wqb[:, 0:1024])
                dma_x(x_cur, 0, pieces=(1,))
                for i in range(1, 4):
                    nc.sync.dma_start(out=wqf[:, i * 1024:(i + 1) * 1024],
                                      in_=wqb[:, i * 1024:(i + 1) * 1024])
                dma_x(x_cur, 0, pieces=(2, 3))
                for w_sb, w_d in ((wk_sb, wkb), (wv_sb, wvb)):
                    nc.sync.dma_start(out=w_sb[:].rearrange("p a b -> p (a b)"),
                                      in_=w_d[:])
                nc.sync.dma_start(out=wo_sb[:].rearrange("p a b -> p (a b)"),
                                  in_=wob[:])

                def rmsnorm_evict(ps, dst, w_col, sl):
                    raw = ev.tile([128, 512], F32, tag="raw", bufs=3)
                    nc.vector.tensor_copy(raw[:], ps[:])
                    sq = ev.tile([128, 512], F32R, tag="sq", bufs=2)
                    nc.scalar.activation(sq[:], ps[:], AF.Square)
                    ps_ss = ps1.tile([1, 512], F32, tag="ss", name="ps_ss",
                                     bufs=2)
                    nc.tensor.matmul(ps_ss[:], ones_col[:], sq[:],
                                     start=True, stop=True,
                                     skip_group_check=True)
                    ms_row = ev.tile([1, 512], F32, tag="msr", bufs=2)
                    nc.scalar.activation(ms_row[:], ps_ss[:], AF.Identity,
                                         bias=eps_sb[:], scale=1.0 / 128.0)
                    rec = ev.tile([1, 512], F32, tag="rec", bufs=2)
                    nc.vector.reciprocal_approx_fast(out=rec[:], in_=ms_row[:])
                    rrms = ev.tile([1, 512], F32R, tag="rrms", bufs=2)
                    nc.scalar.activation(rrms[:], rec[:], AF.Sqrt)
                    rb = ev.tile([128, 512], F32R, tag="rb", bufs=2)
                    nc.gpsimd.partition_broadcast(rb[:], rrms[:])
                    nc.vector.scalar_tensor_tensor(
                        dst[:, sl], raw[:], w_col[:], rb[:],
                        op0=MUL, op1=MUL)

                for n in range(NCH):
                    sl = slice(n * 512, (n + 1) * 512)
                    x_n = x_cur
                    if n + 1 < NCH:
                        x_cur = stream.tile([128, KT_D, 512], BF16, tag="x",
                                            bufs=2)
                        dma_x(x_cur, n + 1)
                    # q/k groups, one PSUM accumulation at a time
                    for w_sb, dst_l, w_col, pstag in (
                        (wq_sb, qT, qw_sb, "psq"),
                        (wk_sb, kT, kw_sb, "psk"),
                    ):
                        for m in range(HPC):
                            ms = slice(m * DH, (m + 1) * DH)
                            ps = ps1.tile([128, 512], F32, tag=f"{pstag}{m}",
                                          name=f"{pstag}{m}")
                            for kt in range(KT_D):
                                nc.tensor.matmul(ps[:], w_sb[:, kt, ms],
                                                 x_n[:, kt, :],
                                                 start=(kt == 0),
                                                 stop=(kt == KT_D - 1),
                                                 skip_group_check=True)
                            rmsnorm_evict(ps, dst_l[m], w_col, sl)
                    # v group: two [128,512] banks, 2 seq subtiles each
                    ps_v = [ps1.tile([128, 512], F32, tag=f"psv{i}",
                                     name=f"psv{i}") for i in range(2)]
                    for kt in range(KT_D):
                        for sm in range(4):
                            pv = ps_v[sm // 2][:, (sm % 2) * 256:
                                               (sm % 2) * 256 + 256]
                            nc.tensor.matmul(pv,
                                             x_n[:, kt, sm * 128:(sm + 1) * 128],
                                             wv_sb[:, kt, :],
                                             start=(kt == 0 and sm % 2 == 0),
                                             stop=(kt == KT_D - 1),
                                             skip_group_check=True)
                    for i in range(2):
                        nc.scalar.copy(
                            v_sb[:, n * 4 + 2 * i:n * 4 + 2 * i + 2, :].rearrange(
                                "p a b -> p (a b)"),
                            ps_v[i][:])

            # ============ Phase 2+3: attention + out-projection ============
            # 1024-wide q blocks; S^T scores span two PSUM banks; sum-exp
            # accumulates in bf16 (DVE 2x mode).  All attention-loop PSUM
            # evictions run on the vector engine; scalar does only exp.
            with (
                tc.tile_pool(name="ps2", bufs=1, space="PSUM") as ps2,
            ):
                NQB = S // 1024

                def outproj_unit(uq_lo, uW, mo, tail=False):
                    # uW-wide out-projection for columns [uq_lo, uq_lo+uW)
                    mosl = slice(mo * 128, (mo + 1) * 128)
                    # in the tail all attention PSUM banks are free: rotate
                    # through se/pss/pso for a 4-deep pipeline that keeps the
                    # matmul stream continuous (and the HAM clock up)
                    tag = ("se", "pss", "pso")[mo % 3] if tail else "se"
                    ps_y = ps2.tile([128, uW], F32, tag=tag, name="ps_y",
                                    bufs=2 if tag == "pss" else 1)
                    for h2 in range(HPC):
                        for u in range(uW // 512):
                            usl = slice(uq_lo + u * 512, uq_lo + (u + 1) * 512)
                            nc.tensor.matmul(ps_y[:, u * 512:(u + 1) * 512],
                                             wo_sb[:, h2, mosl], o_sb[h2][:, usl],
                                             start=(h2 == 0), stop=(h2 == HPC - 1),
                                             skip_group_check=True)
                    y = stream.tile([128, uW], F16, tag="y", bufs=8)
                    if tail and uW == 1024:
                        # both engines are otherwise idle in the tail
                        nc.scalar.activation(y[:, 0:512], ps_y[:, 0:512],
                                             AF.Copy)
                        nc.vector.tensor_copy(y[:, 512:1024], ps_y[:, 512:1024])
                    else:
                        nc.vector.tensor_copy(y[:], ps_y[:])
                    nc.sync.dma_start(out=outT_t[mo][:, uq_lo:uq_lo + uW],
                                      in_=y[:])

                def attn_block(h, q_lo, W, slot_units, end_units):
                    # one softmax-attention block over q columns
                    # [q_lo, q_lo+W); slot_units maps kt -> out-proj units
                    NU = W // 512
                    ps_o = ps2.tile([128, W], F32, tag="pso", bufs=1)
                    acc = ev.tile([128, W], BF16, tag="acc", bufs=2, name="acc")
                    pt_prev = [None]

                    def emit_pv(kt2, pt2):
                        for u in range(NU):
                            nc.tensor.matmul(ps_o[:, u * 512:(u + 1) * 512],
                                             v_sb[:, kt2, h * DH:(h + 1) * DH],
                                             pt2[:, u * 512:(u + 1) * 512],
                                             start=(kt2 == 0),
                                             stop=(kt2 == KT_S - 1),
                                             skip_group_check=True)

                    pv_pend = []
                    for kt in range(KT_S):
                        ksl = slice(kt * 128, (kt + 1) * 128)
                        ps_s = ps2.tile([128, W], F32, tag="pss", bufs=2)
                        for u in range(NU):
                            usl = slice(q_lo + u * 512, q_lo + (u + 1) * 512)
                            nc.tensor.matmul(ps_s[:, u * 512:(u + 1) * 512],
                                             kT[h][:, ksl], qT[h][:, usl],
                                             start=True, stop=True,
                                             skip_group_check=True)
                        pt = stream.tile([128, W], BF16, tag="pt", bufs=8)
                        nc.scalar.activation(pt[:], ps_s[:], AF.Exp, scale=SCALE)
                        if kt % 2 == 0:
                            pt_prev[0] = pt
                        else:
                            pair = ev.tile([128, W], BF16, tag="pair",
                                           bufs=2, name="pair")
                            nc.vector.tensor_add(pair[:], pt_prev[0][:], pt[:])
                            if kt == 1:
                                nc.vector.tensor_copy(acc[:], pair[:])
                            else:
                                nc.vector.tensor_add(acc[:], acc[:], pair[:])
                        pv_pend.append((kt, pt))
                        if len(pv_pend) > 2:
                            emit_pv(*pv_pend.pop(0))
                        for unit in slot_units.get(kt, ()):
                            outproj_unit(*unit)
                    # sum-exp first: its scalar/vector/gpsimd chain runs
                    # while the tensor engine drains PV + out-proj units
                    ps_se = ps2.tile([1, W], F32, tag="se", name="ps_se")
                    for u in range(NU):
                        nc.tensor.matmul(ps_se[:, u * 512:(u + 1) * 512],
                                         ones_col_b[:],
                                         acc[:, u * 512:(u + 1) * 512],
                                         start=True, stop=True,
                                         skip_group_check=True)
                    for kt2, pt2 in pv_pend:
                        emit_pv(kt2, pt2)
                    for unit in end_units:
                        outproj_unit(*unit)
                    # normalization chain pipelined at 512-col granularity so
                    # ps_o frees as early as possible for the next block
                    rec2 = ev.tile([1, W], F32, tag="rec2", bufs=1)
                    rb2 = ev.tile([128, W], F32, tag="rb2", bufs=1)
                    for u in range(NU):
                        usl = slice(u * 512, (u + 1) * 512)
                        nc.vector.reciprocal_approx_fast(
                            out=rec2[:, usl], in_=ps_se[:, usl])
                        nc.gpsimd.partition_broadcast(rb2[:, usl],
                                                      rec2[:, usl])
                    for u in range(NU):
                        usl = slice(u * 512, (u + 1) * 512)
                        nc.vector.tensor_mul(
                            o_sb[h][:, q_lo + u * 512:q_lo + (u + 1) * 512],
                            ps_o[:, usl], rb2[:, usl])

                def spread(units, slots):
                    # deal units round-robin onto kt slots (2 per slot max)
                    m = {}
                    for i, u in enumerate(units):
                        m.setdefault(slots[i % len(slots)], []).append(u)
                    return m

                STD_SLOTS = (3, 7, 11, 15, 19, 23)
                for qb in range(NQB):
                    for h in range(HPC):
                        if qb == 0:
                            attn_block(h, qb * 1024, 1024, {}, ())
                        else:
                            us = [(1024 * (qb - 1), 1024, h * 8 + i)
                                  for i in range(8)]
                            attn_block(h, qb * 1024, 1024,
                                       spread(us, (1, 5, 9, 13, 17, 21, 25,
                                                   29)), ())
                for mo in range(D // 128):
                    outproj_unit((NQB - 1) * 1024, 1024, mo, tail=True)

    nc.compile()
    return nc


_NC_CACHE = None


def _get_nc():
    global _NC_CACHE
    if _NC_CACHE is None:
        _NC_CACHE = build()
    return _NC_CACHE


def _ensure_axon_hooks_stub():
    """bass_utils imports antenv.axon_hooks when tracing is requested via env;
    provide a no-op stub if the image lacks it so a stray BASS_TRACE cannot
    crash the run."""
    import types
    try:
        from antenv import axon_hooks  # noqa: F401
        return
    except Exception:
        pass
    try:
        import antenv
        m = types.ModuleType("antenv.axon_hooks")
        m.set_axon_ntff_profile_hook = lambda h: None
        m.get_axon_ntff_profile_hook = lambda: None
        sys.modules["antenv.axon_hooks"] = m
        antenv.axon_hooks = m
    except Exception:
        pass


def kernel(x, wq, wk, wv, wo, q_norm_w, k_norm_w):
    import ml_dtypes
    from concourse import bass_utils

    _ensure_axon_hooks_stub()

    bf16 = ml_dtypes.bfloat16
    x = np.asarray(x, dtype=np.float32)
    wq = np.asarray(wq, dtype=np.float32)
    wk = np.asarray(wk, dtype=np.float32)
    wv = np.asarray(wv, dtype=np.float32)
    wo = np.asarray(wo, dtype=np.float32)
    q_norm_w = np.asarray(q_norm_w, dtype=np.float32).reshape(DH, 1)
    k_norm_w = np.asarray(k_norm_w, dtype=np.float32).reshape(DH, 1)

    B = x.shape[0]
    x_t = np.ascontiguousarray(x.reshape(S, D).T)           # [D, S]
    # [(n p), (kt c)]: one phase-1 chunk = contiguous 8KB/partition
    x_pack = np.ascontiguousarray(
        x_t.reshape(KT_D, 128, NCH, 512).transpose(2, 1, 0, 3)
    ).reshape(NCH * 128, KT_D * 512).astype(bf16)

    def pack_w(w_core):   # [D, M] -> [128, 16*M] with row d = kt*128 + p
        M = w_core.shape[1]
        return np.ascontiguousarray(
            w_core.reshape(KT_D, 128, M).transpose(1, 0, 2)
        ).reshape(128, KT_D * M).astype(bf16)

    in_maps = []
    for c in range(NC):
        hsl = slice(c * DHC, (c + 1) * DHC)
        wo_core = wo[:, hsl].T                              # [256, D]
        wo_pack = np.ascontiguousarray(
            wo_core.reshape(HPC, 128, D).transpose(1, 0, 2)
        ).reshape(128, HPC * D).astype(bf16)
        in_maps.append({
            "xb": x_pack,
            "wqb": pack_w(wq[hsl, :].T),
            "wkb": pack_w(wk[hsl, :].T),
            "wvb": pack_w(wv[hsl, :].T),
            "wob": wo_pack,
            "qw": q_norm_w,
            "kw": k_norm_w,
            "ones_c": np.ones((128, 1), dtype=np.float32),
        })

    nc = _get_nc()
    res = bass_utils.run_bass_kernel_spmd(
        nc, in_maps, core_ids=list(range(NC)), trace=TRACE,
    )
    acc = res.results[0]["outT"].astype(np.float32)
    for c in range(1, NC):
        acc = acc + res.results[c]["outT"].astype(np.float32)
    out = np.ascontiguousarray(acc.T).reshape(B, S, D)
    if TRACE:
        kernel.last_exec_time_ns = res.exec_time_ns
        kernel.last_results = res
    return out


# revision 46
# speedup vs baseline: 1.0001x; 1.0001x over previous
"""Multi-head self-attention (B=1, S=4096, D=2048, H=16, Dh=128) on 8 TRN2
NeuronCores. Head-sharded tensor parallelism: each core computes 2 heads end to
end in transposed layout, writes its partial out-projection [D, S] as fp16; the
host sums the 8 partials and transposes back to [S, D].

Dtype strategy: activations/weights stream as bf16 (matmul inputs), all matmul
accumulation is fp32 in PSUM.  q/k after rmsnorm are fp16 (same matmul speed,
more mantissa).  Attention scores are computed in S^T layout [k, q] so the
softmax reduction is a ones-column matmul (partition sum) and no transposes are
needed anywhere; the inter-tile sum-exp accumulator is bf16 so every vector add
runs in the DVE's packed 2x mode.

Schedule: phase 1 runs one output group (q0/k0/q1/k1/v) at a time over all 16
k-tiles of a resident x chunk, so each group's PSUM eviction + rmsnorm chain
hides under the next group's matmuls.  Phase 2 interleaves the previous q-block
out-projection into the attention loop; all PSUM evictions in the attention
loop go through the vector engine so the scalar engine does nothing but exp.
"""
import sys
import numpy as np

for _p in ("/opt/trn_rl_repo",):
    if _p not in sys.path:
        sys.path.append(_p)

import concourse.bacc as bacc
import concourse.mybir as mybir
import concourse.tile as tile

F32 = mybir.dt.float32
F32R = mybir.dt.float32r
F16 = mybir.dt.float16
BF16 = mybir.dt.bfloat16
AF = mybir.ActivationFunctionType
MUL = mybir.AluOpType.mult

D = 2048            # d_model
S = 4096            # sequence length
DH = 128            # head dim
HPC = 2             # heads per core
DHC = HPC * DH      # 256 head-dims per core
NC = 8              # cores
EPS = 1e-6
SCALE = 1.0 / np.sqrt(DH)

NCH = S // 512      # 8 seq chunks of 512
KT_D = D // 128     # 16 k-tiles over d_model
KT_S = S // 128     # 32 k-tiles over sequence

TRACE = False       # set by test harness for profiling runs


def build():
    nc = bacc.Bacc("TRN2", target_bir_lowering=False, debug=False)

    # x packed on host as [(n p), (kt c)] so one chunk is a single
    # contiguous 8KB/partition DMA
    xb = nc.dram_tensor("xb", [NCH * 128, KT_D * 512], BF16,
                        kind="ExternalInput")
    wqb = nc.dram_tensor("wqb", [128, KT_D * DHC], BF16, kind="ExternalInput")
    wkb = nc.dram_tensor("wkb", [128, KT_D * DHC], BF16, kind="ExternalInput")
    wvb = nc.dram_tensor("wvb", [128, KT_D * DHC], BF16, kind="ExternalInput")
    wob = nc.dram_tensor("wob", [128, HPC * D], BF16, kind="ExternalInput")
    qw = nc.dram_tensor("qw", [DH, 1], F32, kind="ExternalInput")
    kw = nc.dram_tensor("kw", [DH, 1], F32, kind="ExternalInput")
    ones_c_d = nc.dram_tensor("ones_c", [128, 1], F32, kind="ExternalInput")
    outT = nc.dram_tensor("outT", [D, S], F16, kind="ExternalOutput")

    xb_t = xb.rearrange("(n p) r -> n p r", p=128)          # [8,128,8192]
    outT_t = outT.rearrange("(mo p) s -> mo p s", p=128)    # [16,128,4096]

    with tile.TileContext(nc) as tc, \
         nc.allow_low_precision(reason="bf16/fp16/f32r compute is intentional"):
        with (
            tc.tile_pool(name="consts", bufs=1) as consts,
            tc.tile_pool(name="big", bufs=1) as big,
            tc.tile_pool(name="stream", bufs=6) as stream,
            tc.tile_pool(name="ev", bufs=1) as ev,
        ):
            # ---- residents ----
            # const tiles; their DMAs are deferred until after the first
            # x/wq pieces so they don't delay the first matmul's inputs
            ones_col = consts.tile([128, 1], F32R)         # lhsT for partition-sum
            ones_col_b = consts.tile([128, 1], BF16, tag="onesb")
            nc.vector.memset(ones_col_b[:], 1.0)
            eps_sb = consts.tile([1, 1], F32, tag="eps")
            nc.vector.memset(eps_sb[:], EPS)
            qw_sb = consts.tile([DH, 1], F32, tag="qw")    # per-partition norm w
            kw_sb = consts.tile([DH, 1], F32, tag="kw")

            qT = [big.tile([128, S], F16, tag=f"q{h}", name=f"qT{h}")
                  for h in range(HPC)]
            kT = [big.tile([128, S], F16, tag=f"k{h}", name=f"kT{h}")
                  for h in range(HPC)]
            v_sb = big.tile([128, KT_S, DHC], BF16, tag="v")
            o_sb = [big.tile([128, S], BF16, tag=f"o{h}", name=f"o{h}")
                    for h in range(HPC)]
            wo_sb = big.tile([128, HPC, D], BF16, tag="wo")

            # ========== Phase 1: q/k/v projections + q/k rmsnorm ==========
            # x chunk resident in SBUF; one output group at a time over all
            # 16 k-tiles so each group's eviction chain overlaps the next
            # group's matmuls.
            with (
                tc.tile_pool(name="wqk", bufs=1) as wqk,
                tc.tile_pool(name="ps1", bufs=1, space="PSUM") as ps1,
            ):
                # first x chunk before the weights so the first matmul can
                # start as soon as x(0)+wq land; quarter-granular pieces so
                # the kt=0 matmuls only wait on the first piece
                def dma_x(x_tile, n, pieces=range(4)):
                    xf = x_tile[:].rearrange("p a b -> p (a b)")
                    for i in pieces:
                        qs = slice(i * 2048, (i + 1) * 2048)
                        nc.sync.dma_start(out=xf[:, qs], in_=xb_t[n][:, qs])

                x_cur = stream.tile([128, KT_D, 512], BF16, tag="x", bufs=2)
                wq_sb = wqk.tile([128, KT_D, DHC], BF16, tag="wq")
                wk_sb = wqk.tile([128, KT_D, DHC], BF16, tag="wk")
                wv_sb = wqk.tile([128, KT_D, DHC], BF16, tag="wv")
                dma_x(x_cur, 0, pieces=(0,))
                wqf = wq_sb[:].rearrange("p a b -> p (a b)")
                nc.sync.dma_start(out=wqf[:, 0:1024], in_=wqb[:, 0:1024])
                dma_x(x_cur, 0, pieces=(1,))
                for i in range(1, 4):
                    nc.sync.dma_start(out=wqf[:, i * 1024:(i + 1) * 1024],
                                      in_=wqb[:, i * 1024:(i + 1) * 1024])
                dma_x(x_cur, 0, pieces=(2, 3))
                nc.sync.dma_start(out=ones_col[:], in_=ones_c_d[:].bitcast(F32R))
                nc.sync.dma_start(out=qw_sb[:], in_=qw[:])
                nc.sync.dma_start(out=kw_sb[:], in_=kw[:])
                for w_sb, w_d in ((wk_sb, wkb), (wv_sb, wvb)):
                    nc.sync.dma_start(out=w_sb[:].rearrange("p a b -> p (a b)"),
                                      in_=w_d[:])
                nc.sync.dma_start(out=wo_sb[:].rearrange("p a b -> p (a b)"),
                                  in_=wob[:])

                def rmsnorm_evict(ps, dst, w_col, sl):
                    raw = ev.tile([128, 512], F32, tag="raw", bufs=3)
                    nc.vector.tensor_copy(raw[:], ps[:])
                    sq = ev.tile([128, 512], F32R, tag="sq", bufs=2)
                    nc.scalar.activation(sq[:], ps[:], AF.Square)
                    ps_ss = ps1.tile([1, 512], F32, tag="ss", name="ps_ss",
                                     bufs=2)
                    nc.tensor.matmul(ps_ss[:], ones_col[:], sq[:],
                                     start=True, stop=True,
                                     skip_group_check=True)
                    ms_row = ev.tile([1, 512], F32, tag="msr", bufs=2)
                    nc.scalar.activation(ms_row[:], ps_ss[:], AF.Identity,
                                         bias=eps_sb[:], scale=1.0 / 128.0)
                    rec = ev.tile([1, 512], F32, tag="rec", bufs=2)
                    nc.vector.reciprocal_approx_fast(out=rec[:], in_=ms_row[:])
                    rrms = ev.tile([1, 512], F32R, tag="rrms", bufs=2)
                    nc.scalar.activation(rrms[:], rec[:], AF.Sqrt)
                    rb = ev.tile([128, 512], F32R, tag="rb", bufs=2)
                    nc.gpsimd.partition_broadcast(rb[:], rrms[:])
                    nc.vector.scalar_tensor_tensor(
                        dst[:, sl], raw[:], w_col[:], rb[:],
                        op0=MUL, op1=MUL)

                for n in range(NCH):
                    sl = slice(n * 512, (n + 1) * 512)
                    x_n = x_cur
                    if n + 1 < NCH:
                        x_cur = stream.tile([128, KT_D, 512], BF16, tag="x",
                                            bufs=2)
                        dma_x(x_cur, n + 1)
                    # q/k groups, one PSUM accumulation at a time
                    for w_sb, dst_l, w_col, pstag in (
                        (wq_sb, qT, qw_sb, "psq"),
                        (wk_sb, kT, kw_sb, "psk"),
                    ):
                        for m in range(HPC):
                            ms = slice(m * DH, (m + 1) * DH)
                            ps = ps1.tile([128, 512], F32, tag=f"{pstag}{m}",
                                          name=f"{pstag}{m}")
                            for kt in range(KT_D):
                                nc.tensor.matmul(ps[:], w_sb[:, kt, ms],
                                                 x_n[:, kt, :],
                                                 start=(kt == 0),
                                                 stop=(kt == KT_D - 1),
                                                 skip_group_check=True)
                            rmsnorm_evict(ps, dst_l[m], w_col, sl)
                    # v group: two [128,512] banks, 2 seq subtiles each
                    ps_v = [ps1.tile([128, 512], F32, tag=f"psv{i}",
                                     name=f"psv{i}") for i in range(2)]
                    for kt in range(KT_D):
                        for sm in range(4):
                            pv = ps_v[sm // 2][:, (sm % 2) * 256:
                                               (sm % 2) * 256 + 256]
                            nc.tensor.matmul(pv,
                                             x_n[:, kt, sm * 128:(sm + 1) * 128],
                                             wv_sb[:, kt, :],
                                             start=(kt == 0 and sm % 2 == 0),
                                             stop=(kt == KT_D - 1),
                                             skip_group_check=True)
                    for i in range(2):
                        nc.scalar.copy(
                            v_sb[:, n * 4 + 2 * i:n * 4 + 2 * i + 2, :].rearrange(
                                "p a b -> p (a b)"),
                            ps_v[i][:])

            # ============ Phase 2+3: attention + out-projection ============
            # 1024-wide q blocks; S^T scores span two PSUM banks; sum-exp
            # accumulates in bf16 (DVE 2x mode).  All attention-loop PSUM
            # evictions run on the vector engine; scalar does only exp.
            with (
                tc.tile_pool(name="ps2", bufs=1, space="PSUM") as ps2,
            ):
                NQB = S // 1024

                def outproj_unit(uq_lo, uW, mo, tail=False):
                    # uW-wide out-projection for columns [uq_lo, uq_lo+uW)
                    mosl = slice(mo * 128, (mo + 1) * 128)
                    # in the tail all attention PSUM banks are free: rotate
                    # through se/pss/pso for a 4-deep pipeline that keeps the
                    # matmul stream continuous (and the HAM clock up)
                    tag = ("se", "pss", "pso")[mo % 3] if tail else "se"
                    ps_y = ps2.tile([128, uW], F32, tag=tag, name="ps_y",
                                    bufs=2 if tag == "pss" else 1)
                    for h2 in range(HPC):
                        for u in range(uW // 512):
                            usl = slice(uq_lo + u * 512, uq_lo + (u + 1) * 512)
                            nc.tensor.matmul(ps_y[:, u * 512:(u + 1) * 512],
                                             wo_sb[:, h2, mosl], o_sb[h2][:, usl],
                                             start=(h2 == 0), stop=(h2 == HPC - 1),
                                             skip_group_check=True)
                    y = stream.tile([128, uW], F16, tag="y", bufs=8)
                    if tail and uW == 1024:
                        # both engines are otherwise idle in the tail
                        nc.scalar.activation(y[:, 0:512], ps_y[:, 0:512],
                                             AF.Copy)
                        nc.vector.tensor_copy(y[:, 512:1024], ps_y[:, 512:1024])
                    else:
                        nc.vector.tensor_copy(y[:], ps_y[:])
                    nc.sync.dma_start(out=outT_t[mo][:, uq_lo:uq_lo + uW],
                                      in_=y[:])

                def attn_block(h, q_lo, W, slot_units, end_units):
                    # one softmax-attention block over q columns
                    # [q_lo, q_lo+W); slot_units maps kt -> out-proj units
                    NU = W // 512
                    ps_o = ps2.tile([128, W], F32, tag="pso", bufs=1)
                    acc = ev.tile([128, W], BF16, tag="acc", bufs=2, name="acc")
                    pt_prev = [None]

                    def emit_pv(kt2, pt2):
                        for u in range(NU):
                            nc.tensor.matmul(ps_o[:, u * 512:(u + 1) * 512],
                                             v_sb[:, kt2, h * DH:(h + 1) * DH],
                                             pt2[:, u * 512:(u + 1) * 512],
                                             start=(kt2 == 0),
                                             stop=(kt2 == KT_S - 1),
                                             skip_group_check=True)

                    pv_pend = []
                    for kt in range(KT_S):
                        ksl = slice(kt * 128, (kt + 1) * 128)
                        ps_s = ps2.tile([128, W], F32, tag="pss", bufs=2)
                        for u in range(NU):
                            usl = slice(q_lo + u * 512, q_lo + (u + 1) * 512)
                            nc.tensor.matmul(ps_s[:, u * 512:(u + 1) * 512],
                                             kT[h][:, ksl], qT[h][:, usl],
                                             start=True, stop=True,
                                             skip_group_check=True)
                        pt = stream.tile([128, W], BF16, tag="pt", bufs=8)
                        nc.scalar.activation(pt[:], ps_s[:], AF.Exp, scale=SCALE)
                        if kt % 2 == 0:
                            pt_prev[0] = pt
                        else:
                            pair = ev.tile([128, W], BF16, tag="pair",
                                           bufs=2, name="pair")
                            nc.vector.tensor_add(pair[:], pt_prev[0][:], pt[:])
                            if kt == 1:
                                nc.vector.tensor_copy(acc[:], pair[:])
                            else:
                                nc.vector.tensor_add(acc[:], acc[:], pair[:])
                        pv_pend.append((kt, pt))
                        if len(pv_pend) > 2:
                            emit_pv(*pv_pend.pop(0))
                        for unit in slot_units.get(kt, ()):
                            outproj_unit(*unit)
                    # sum-exp first: its scalar/vector/gpsimd chain runs
                    # while the tensor engine drains PV + out-proj units
                    ps_se = ps2.tile([1, W], F32, tag="se", name="ps_se")
                    for u in range(NU):
                        nc.tensor.matmul(ps_se[:, u * 512:(u + 1) * 512],
                                         ones_col_b[:],
                                         acc[:, u * 512:(u + 1) * 512],
                                         start=True, stop=True,
                                         skip_group_check=True)
                    for kt2, pt2 in pv_pend:
                        emit_pv(kt2, pt2)
                    for unit in end_units:
                        outproj_unit(*unit)
                    # normalization chain pipelined at 512-col granularity so
                    # ps_o frees as early as possible for the next block
                    rec2 = ev.tile([1, W], F32, tag="rec2", bufs=1)
                    rb2 = ev.tile([128, W], F32, tag="rb2", bufs=1)
                    for u in range(NU):
                        usl = slice(u * 512, (u + 1) * 512)
                        nc.vector.reciprocal_approx_fast(
                            out=rec2[:, usl], in_=ps_se[:, usl])
                        nc.gpsimd.partition_broadcast(rb2[:, usl],
                                                      rec2[:, usl])
                    for u in range(NU):
                        usl = slice(u * 512, (u + 1) * 512)
                        nc.vector.tensor_mul(
                            o_sb[h][:, q_lo + u * 512:q_lo + (u + 1) * 512],
                            ps_o[:, usl], rb2[:, usl])

                def spread(units, slots):
                    # deal units round-robin onto kt slots (2 per slot max)
                    m = {}
                    for i, u in enumerate(units):
                        m.setdefault(slots[i % len(slots)], []).append(u)
                    return m

                STD_SLOTS = (3, 7, 11, 15, 19, 23)
                for qb in range(NQB):
                    for h in range(HPC):
                        if qb == 0:
                            attn_block(h, qb * 1024, 1024, {}, ())
                        else:
                            us = [(1024 * (qb - 1), 1024, h * 8 + i)
                                  for i in range(8)]
                            attn_block(h, qb * 1024, 1024,
                                       spread(us, (3, 7, 11, 15, 19, 23, 27,
                                                   30)), ())
                for mo in range(D // 128):
                    outproj_unit((NQB - 1) * 1024, 1024, mo, tail=True)

    nc.compile()
    return nc


_NC_CACHE = None


def _get_nc():
    global _NC_CACHE
    if _NC_CACHE is None:
        _NC_CACHE = build()
    return _NC_CACHE


def _ensure_axon_hooks_stub():
    """bass_utils imports antenv.axon_hooks when tracing is requested via env;
    provide a no-op stub if the image lacks it so a stray BASS_TRACE cannot
    crash the run."""
    import types
    try:
        from antenv import axon_hooks  # noqa: F401
        return
    except Exception:
        pass
    try:
        import antenv
        m = types.ModuleType("antenv.axon_hooks")
        m.set_axon_ntff_profile_hook = lambda h: None
        m.get_axon_ntff_profile_hook = lambda: None
        sys.modules["antenv.axon_hooks"] = m
        antenv.axon_hooks = m
    except Exception:
        pass


def kernel(x, wq, wk, wv, wo, q_norm_w, k_norm_w):
    import ml_dtypes
    from concourse import bass_utils

    _ensure_axon_hooks_stub()

    bf16 = ml_dtypes.bfloat16
    x = np.asarray(x, dtype=np.float32)
    wq = np.asarray(wq, dtype=np.float32)
    wk = np.asarray(wk, dtype=np.float32)
    wv = np.asarray(wv, dtype=np.float32)
    wo = np.asarray(wo, dtype=np.float32)
    q_norm_w = np.asarray(q_norm_w, dtype=np.float32).reshape(DH, 1)
    k_norm_w = np.asarray(k_norm_w, dtype=np.float32).reshape(DH, 1)

    B = x.shape[0]
    x_t = np.ascontiguousarray(x.reshape(S, D).T)           # [D, S]
    # [(n p), (kt c)]: one phase-1 chunk = contiguous 8KB/partition
    x_pack = np.ascontiguousarray(
        x_t.reshape(KT_D, 128, NCH, 512).transpose(2, 1, 0, 3)
    ).reshape(NCH * 128, KT_D * 512).astype(bf16)

    def pack_w(w_core):   # [D, M] -> [128, 16*M] with row d = kt*128 + p
        M = w_core.shape[1]
        return np.ascontiguousarray(
            w_core.reshape(KT_D, 128, M).transpose(1, 0, 2)
        ).reshape(128, KT_D * M).astype(bf16)

    in_maps = []
    for c in range(NC):
        hsl = slice(c * DHC, (c + 1) * DHC)
        wo_core = wo[:, hsl].T                              # [256, D]
        wo_pack = np.ascontiguousarray(
            wo_core.reshape(HPC, 128, D).transpose(1, 0, 2)
        ).reshape(128, HPC * D).astype(bf16)
        in_maps.append({
            "xb": x_pack,
            "wqb": pack_w(wq[hsl, :].T),
            "wkb": pack_w(wk[hsl, :].T),
            "wvb": pack_w(wv[hsl, :].T),
            "wob": wo_pack,
            "qw": q_norm_w,
            "kw": k_norm_w,
            "ones_c": np.ones((128, 1), dtype=np.float32),
        })

    nc = _get_nc()
    res = bass_utils.run_bass_kernel_spmd(
        nc, in_maps, core_ids=list(range(NC)), trace=TRACE,
    )
    acc = res.results[0]["outT"].astype(np.float32)
    for c in range(1, NC):
        acc = acc + res.results[c]["outT"].astype(np.float32)
    out = np.ascontiguousarray(acc.T).reshape(B, S, D)
    if TRACE:
        kernel.last_exec_time_ns = res.exec_time_ns
        kernel.last_results = res
    return out
